# revision 11
# baseline (speedup 1.0000x reference)
"""Trainium2 Bass kernel for nn_ExtractPatchesPositionLayer.

Reference semantics: per image b, bilinear-translate the (522,522,1) padded
object by t = -positions[b] (tfa.translate: out(y,x) = img(y+py, x+px),
zero fill outside), then center-crop 5px -> (512,512,1).

The shift is constant per image, so floor/frac of the offset give an integer
window start (A,B) plus four bilinear corner weights c00,c01,c10,c11. The
host extracts each image's integer-aligned 513x513 window (zero-padded at the
borders, row-padded to 514 for even alignment) and casts it to fp16 — after
that every device access pattern is STATIC, so all DMAs are plain HWDGE
copies that spray evenly across all 16 SDMA engines (dynamic-offset DMAs all
serialize on one engine/queue, which was the original 1.4 ms bottleneck).

Blocked layout: SBUF partition p holds 5 consecutive window rows (4 output
rows + 1 halo row, the re-read is nearly free — same DRAM rows) contiguous in
DRAM -> ~5 KB load packets, and BOTH bilinear taps become free-dim shifts of
the same tile:

    out[p, k, j] = c00*w[p, k*RS+j]   + c01*w[p, k*RS+j+1]
                 + c10*w[p,(k+1)*RS+j] + c11*w[p,(k+1)*RS+j+1]

which the (otherwise idle) tensor engine evaluates as 4 accumulating matmuls
per 512-wide chunk with SCALED-IDENTITY stationary weights (lhsT = c_ij * I):
out = sum_ij (c_ij I)^T @ shifted_view(w). PSUM accumulates in fp32; the
result is rounded once to fp16 for the store (output HBM traffic halves; the
host upcasts to fp32 — total rel err ~6e-4, far under the 2e-2 gate).
DVE/ACT only build the tiny scaled identities and copy/round PSUM->SBUF.
Sharding: batch 256 -> 32 images x 8 cores, embarrassingly parallel.
"""

from dataclasses import dataclass

import numpy as np

import concourse.bacc as bacc
import concourse.bass as bass
import concourse.mybir as mybir
import concourse.tile as tile
from concourse.bass_utils import run_bass_kernel_spmd

PAD = 5


@dataclass(frozen=True)
class Cfg:
    bpc: int   # images per core
    n: int     # output height/width (512)

    @property
    def win(self):  # window rows/cols actually used
        return self.n + 1

    @property
    def rs(self):   # row stride in the staged window (win padded to even)
        return self.win + 1

    @property
    def rpp(self):  # output rows per partition
        return self.n // 128


def build_nc(cfg: Cfg) -> bass.Bass:
    BPC, N, RS = cfg.bpc, cfg.n, cfg.rs
    K = cfg.rpp                 # 4 output rows per partition
    IMG = cfg.win * RS          # elems per staged image (513*514)
    NN = N * N                  # elems per output image
    f16 = mybir.dt.float16
    f32 = mybir.dt.float32

    nc = bacc.Bacc("TRN2", target_bir_lowering=False, debug=False)
    x_d = nc.declare_dram_parameter("x", [BPC, IMG], f16, isOutput=False)
    wm_d = nc.declare_dram_parameter("wm", [128, BPC * 4], f32, isOutput=False)
    id_d = nc.declare_dram_parameter("idm", [128, 128], f16, isOutput=False)
    y_d = nc.declare_dram_parameter("y", [BPC, NN], f16, isOutput=True)

    with tile.TileContext(nc) as tc:
        with (
            tc.tile_pool(name="const", bufs=1) as constp,
            tc.tile_pool(name="win", bufs=4) as winp,
            tc.tile_pool(name="lt", bufs=4) as ltp,
            tc.tile_pool(name="outp", bufs=4) as outp,
            tc.tile_pool(name="ps", bufs=2, space="PSUM") as psp,
        ):
            wm_sb = constp.tile([128, BPC * 4], f32, tag="wm")
            nc.sync.dma_start(wm_sb[:], wm_d[:, :])
            id_sb = constp.tile([128, 128], f16, tag="idm")
            nc.sync.dma_start(id_sb[:], id_d[:, :])

            for b in range(BPC):
                # partition p <- window rows K*p .. K*p+K (halo row shared
                # with partition p+1); contiguous 5*RS-elem read per partition
                w = winp.tile([128, (K + 1) * RS], f16, tag="w")
                nc.sync.dma_start(
                    w[:], bass.AP(x_d, b * IMG, [[K * RS, 128], [1, (K + 1) * RS]])
                )

                # stationary weights: lhsT_ij = c_ij * I  (tiny DVE muls)
                lts = []
                for ij in range(4):
                    lt = ltp.tile([128, 128], f16, tag=f"lt{ij}")
                    nc.vector.tensor_scalar_mul(
                        lt[:], id_sb[:], wm_sb[:, 4 * b + ij: 4 * b + ij + 1])
                    lts.append(lt)

                # 4 shifted taps x 4 chunks; grouped by lhsT so the PE keeps
                # each weight matrix loaded for 4 consecutive matmuls
                ps = psp.tile([128, K * N], f32, tag="ps")
                shift = [0, 1, RS, RS + 1]
                for ij in range(4):
                    for k in range(K):
                        nc.tensor.matmul(
                            out=ps[:, k * N:(k + 1) * N],
                            lhsT=lts[ij][:],
                            rhs=w[:, k * RS + shift[ij]: k * RS + shift[ij] + N],
                            start=(ij == 0), stop=(ij == 3))

                # PSUM -> SBUF with a single fp32->fp16 rounding, split
                # across DVE and ACT
                o = outp.tile([128, K * N], f16, tag="o")
                half = K * N // 2
                nc.vector.tensor_copy(o[:, 0:half], ps[:, 0:half])
                nc.scalar.copy(o[:, half:], ps[:, half:])
                # partition p -> output rows K*p .. K*p+K-1 (4 KB contiguous)
                nc.scalar.dma_start(
                    bass.AP(y_d, b * NN, [[K * N, 128], [1, K * N]]), o[:])
    nc.compile()
    return nc


def host_prep(padded: np.ndarray, positions: np.ndarray, n_cores: int):
    """Shard + stage integer-aligned fp16 windows.

    padded: (B, npad, npad) f32, positions: (B, 2)."""
    B, npad, _ = padded.shape
    n = npad - 2 * PAD
    cfg = Cfg(bpc=B // n_cores, n=n)
    win, rs = cfg.win, cfg.rs

    px = positions[:, 0].astype(np.float64)
    py = positions[:, 1].astype(np.float64)
    fy = np.floor(py)
    fx = np.floor(px)
    ay = (PAD + fy).astype(np.int64)
    ax = (PAD + fx).astype(np.int64)
    wy = (py - fy).astype(np.float32)
    wx = (px - fx).astype(np.float32)

    xw = np.zeros((B, win, rs), dtype=np.float16)
    for b in range(B):
        r0 = max(int(ay[b]), 0)
        r1 = min(int(ay[b]) + win, npad)
        c0 = max(int(ax[b]), 0)
        c1 = min(int(ax[b]) + win, npad)
        if r1 > r0 and c1 > c0:
            xw[b, r0 - ay[b]:r1 - ay[b], c0 - ax[b]:c1 - ax[b]] = \
                padded[b, r0:r1, c0:c1]

    bpc = cfg.bpc
    idm = np.eye(128, dtype=np.float16)
    in_maps = []
    for cidx in range(n_cores):
        sl = slice(cidx * bpc, (cidx + 1) * bpc)
        wmat = np.empty((128, bpc * 4), dtype=np.float32)
        wmat[:, 0::4] = ((1 - wy[sl]) * (1 - wx[sl]))[None, :]  # c00: no shift
        wmat[:, 1::4] = ((1 - wy[sl]) * wx[sl])[None, :]        # c01: +1 col
        wmat[:, 2::4] = (wy[sl] * (1 - wx[sl]))[None, :]        # c10: +1 row
        wmat[:, 3::4] = (wy[sl] * wx[sl])[None, :]              # c11: both
        in_maps.append({
            "x": xw[sl].reshape(bpc, win * rs),
            "wm": wmat,
            "idm": idm,
        })
    return cfg, in_maps


N_CORES = 8
_nc_cache: dict = {}


def kernel(padded_obj: np.ndarray, positions: np.ndarray) -> np.ndarray:
    padded_obj = np.asarray(padded_obj)
    positions = np.asarray(positions)
    B, npad, _, C = padded_obj.shape
    cfg, in_maps = host_prep(
        padded_obj.reshape(B, npad, npad).astype(np.float32, copy=False),
        positions, N_CORES)

    nc = _nc_cache.get(cfg)
    if nc is None:
        nc = build_nc(cfg)
        _nc_cache[cfg] = nc

    res = run_bass_kernel_spmd(nc, in_maps, core_ids=list(range(N_CORES)))
    out = np.concatenate([r["y"] for r in res.results], axis=0)
    return out.reshape(B, cfg.n, cfg.n, 1).astype(np.float32)


# revision 12
# speedup vs baseline: 1.1181x; 1.1181x over previous
"""Trainium2 Bass kernel for nn_ExtractPatchesPositionLayer.

Reference semantics: per image b, bilinear-translate the (522,522,1) padded
object by t = -positions[b] (tfa.translate: out(y,x) = img(y+py, x+px),
zero fill outside), then center-crop 5px -> (512,512,1).

The shift is constant per image, so floor/frac of the offset give an integer
window start (A,B) plus four bilinear corner weights c00,c01,c10,c11. The
host extracts each image's integer-aligned 513x513 window (zero-padded at the
borders, row-padded to 514 for even alignment) and casts it to fp16 — after
that every device access pattern is STATIC, so all DMAs are plain HWDGE
copies that spray evenly across all 16 SDMA engines (dynamic-offset DMAs all
serialize on one engine/queue, which was the original 1.4 ms bottleneck).

Blocked layout: SBUF partition p holds 5 consecutive window rows (4 output
rows + 1 halo row, the re-read is nearly free — same DRAM rows) contiguous in
DRAM -> ~5 KB load packets, and BOTH bilinear taps become free-dim shifts of
the same tile:

    out[p, k, j] = c00*w[p, k*RS+j]   + c01*w[p, k*RS+j+1]
                 + c10*w[p,(k+1)*RS+j] + c11*w[p,(k+1)*RS+j+1]

which the (otherwise idle) tensor engine evaluates as 4 accumulating matmuls
per 512-wide chunk with SCALED-IDENTITY stationary weights (lhsT = c_ij * I):
out = sum_ij (c_ij I)^T @ shifted_view(w). PSUM accumulates in fp32; the
result is rounded once to fp16 for the store (output HBM traffic halves; the
host upcasts to fp32 — total rel err ~6e-4, far under the 2e-2 gate).
DVE/ACT only build the tiny scaled identities and copy/round PSUM->SBUF.
Sharding: batch 256 -> 32 images x 8 cores, embarrassingly parallel.
"""

from dataclasses import dataclass

import numpy as np

import concourse.bacc as bacc
import concourse.bass as bass
import concourse.mybir as mybir
import concourse.tile as tile
from concourse.bass_utils import run_bass_kernel_spmd

PAD = 5


@dataclass(frozen=True)
class Cfg:
    bpc: int   # images per core
    n: int     # output height/width (512)

    @property
    def win(self):  # window rows/cols actually used
        return self.n + 1

    @property
    def rs(self):   # row stride in the staged window (win padded to even)
        return self.win + 1

    @property
    def rpp(self):  # output rows per partition
        return self.n // 128


def build_nc(cfg: Cfg) -> bass.Bass:
    BPC, N, RS = cfg.bpc, cfg.n, cfg.rs
    K = cfg.rpp                 # 4 output rows per partition
    IMG = cfg.win * RS          # elems per staged image (513*514)
    NN = N * N                  # elems per output image
    f16 = mybir.dt.float16
    f32 = mybir.dt.float32

    nc = bacc.Bacc("TRN2", target_bir_lowering=False, debug=False)
    x_d = nc.declare_dram_parameter("x", [BPC, IMG], f16, isOutput=False)
    wm_d = nc.declare_dram_parameter("wm", [128, BPC * 4], f32, isOutput=False)
    id_d = nc.declare_dram_parameter("idm", [128, 128], f16, isOutput=False)
    y_d = nc.declare_dram_parameter("y", [BPC, NN], f16, isOutput=True)

    with tile.TileContext(nc) as tc:
        with (
            tc.tile_pool(name="const", bufs=1) as constp,
            tc.tile_pool(name="win", bufs=3) as winp,
            tc.tile_pool(name="lt", bufs=2) as ltp,
            tc.tile_pool(name="outp", bufs=3) as outp,
            tc.tile_pool(name="ps", bufs=2, space="PSUM") as psp,
        ):
            wm_sb = constp.tile([128, BPC * 4], f32, tag="wm")
            nc.sync.dma_start(wm_sb[:], wm_d[:, :])
            id_sb = constp.tile([128, 128], f16, tag="idm")
            nc.sync.dma_start(id_sb[:], id_d[:, :])

            for b in range(BPC):
                # partition p <- window rows K*p .. K*p+K (halo row shared
                # with partition p+1); contiguous 5*RS-elem read per partition
                w = winp.tile([128, (K + 1) * RS], f16, tag="w")
                nc.sync.dma_start(
                    w[:], bass.AP(x_d, b * IMG, [[K * RS, 128], [1, (K + 1) * RS]])
                )

                # stationary weights: lhsT_ij = c_ij * I  (tiny DVE muls)
                lts = []
                for ij in range(4):
                    lt = ltp.tile([128, 128], f16, tag=f"lt{ij}")
                    nc.vector.tensor_scalar_mul(
                        lt[:], id_sb[:], wm_sb[:, 4 * b + ij: 4 * b + ij + 1])
                    lts.append(lt)

                # 4 shifted taps x 4 chunks; grouped by lhsT so the PE keeps
                # each weight matrix loaded for 4 consecutive matmuls
                ps = psp.tile([128, K * N], f32, tag="ps")
                shift = [0, 1, RS, RS + 1]
                for ij in range(4):
                    for k in range(K):
                        nc.tensor.matmul(
                            out=ps[:, k * N:(k + 1) * N],
                            lhsT=lts[ij][:],
                            rhs=w[:, k * RS + shift[ij]: k * RS + shift[ij] + N],
                            start=(ij == 0), stop=(ij == 3))

                # PSUM -> SBUF with a single fp32->fp16 rounding, split
                # across DVE and ACT
                o = outp.tile([128, K * N], f16, tag="o")
                half = K * N // 2
                nc.vector.tensor_copy(o[:, 0:half], ps[:, 0:half])
                nc.scalar.copy(o[:, half:], ps[:, half:])
                # partition p -> output rows K*p .. K*p+K-1 (4 KB contiguous)
                nc.scalar.dma_start(
                    bass.AP(y_d, b * NN, [[K * N, 128], [1, K * N]]), o[:])
    nc.compile()
    return nc


def host_prep(padded: np.ndarray, positions: np.ndarray, n_cores: int):
    """Shard + stage integer-aligned fp16 windows.

    padded: (B, npad, npad) f32, positions: (B, 2)."""
    B, npad, _ = padded.shape
    n = npad - 2 * PAD
    cfg = Cfg(bpc=B // n_cores, n=n)
    win, rs = cfg.win, cfg.rs

    px = positions[:, 0].astype(np.float64)
    py = positions[:, 1].astype(np.float64)
    fy = np.floor(py)
    fx = np.floor(px)
    ay = (PAD + fy).astype(np.int64)
    ax = (PAD + fx).astype(np.int64)
    wy = (py - fy).astype(np.float32)
    wx = (px - fx).astype(np.float32)

    xw = np.zeros((B, win, rs), dtype=np.float16)
    for b in range(B):
        r0 = max(int(ay[b]), 0)
        r1 = min(int(ay[b]) + win, npad)
        c0 = max(int(ax[b]), 0)
        c1 = min(int(ax[b]) + win, npad)
        if r1 > r0 and c1 > c0:
            xw[b, r0 - ay[b]:r1 - ay[b], c0 - ax[b]:c1 - ax[b]] = \
                padded[b, r0:r1, c0:c1]

    bpc = cfg.bpc
    idm = np.eye(128, dtype=np.float16)
    in_maps = []
    for cidx in range(n_cores):
        sl = slice(cidx * bpc, (cidx + 1) * bpc)
        wmat = np.empty((128, bpc * 4), dtype=np.float32)
        wmat[:, 0::4] = ((1 - wy[sl]) * (1 - wx[sl]))[None, :]  # c00: no shift
        wmat[:, 1::4] = ((1 - wy[sl]) * wx[sl])[None, :]        # c01: +1 col
        wmat[:, 2::4] = (wy[sl] * (1 - wx[sl]))[None, :]        # c10: +1 row
        wmat[:, 3::4] = (wy[sl] * wx[sl])[None, :]              # c11: both
        in_maps.append({
            "x": xw[sl].reshape(bpc, win * rs),
            "wm": wmat,
            "idm": idm,
        })
    return cfg, in_maps


N_CORES = 8
_nc_cache: dict = {}


def kernel(padded_obj: np.ndarray, positions: np.ndarray) -> np.ndarray:
    padded_obj = np.asarray(padded_obj)
    positions = np.asarray(positions)
    B, npad, _, C = padded_obj.shape
    cfg, in_maps = host_prep(
        padded_obj.reshape(B, npad, npad).astype(np.float32, copy=False),
        positions, N_CORES)

    nc = _nc_cache.get(cfg)
    if nc is None:
        nc = build_nc(cfg)
        _nc_cache[cfg] = nc

    res = run_bass_kernel_spmd(nc, in_maps, core_ids=list(range(N_CORES)))
    out = np.concatenate([r["y"] for r in res.results], axis=0)
    return out.reshape(B, cfg.n, cfg.n, 1).astype(np.float32)
